# revision 2
# baseline (speedup 1.0000x reference)
"""2x2 average pool + per-channel affine on 8 TRN2 NeuronCores.

Problem: x (16, 64, 512, 512) f32 -> out (16, 64, 256, 256) f32
  out[b,c,i,j] = weight[c] * mean(x[b,c,2i:2i+2,2j:2j+2]) + bias[c]

Sharding: pure data parallel over batch. Core k gets batches [2k, 2k+1]
(128 images of 512x512 per core), weight/bias replicated.

Layout: partition p = (b_local*64 + c) -> one image per partition. The
host pre-transposes each core's shard to [n_iters, P, chunk] so every
load DMA reads ONE dense span of DRAM (4 MiB) instead of 128 segments
strided by 1 MiB — measured 328 vs 285 GB/s/core with 8 cores running
(the 8 cores together sit on the device's aggregate HBM limit, ~2.6
TB/s). The output is likewise written to a [n_iters, P, out_chunk]
dense layout and un-shuffled on the host after the gather.

The output keeps the natural [P, OUT_IMG] layout (a dense
iteration-major output layout measured identical same-window).

Per iteration: one 4 MiB load (alternating between the two HWDGE rings
SP/ACT), vertical 2:1 pool with one tensor_tensor add (row pairs are
adjacent in the free dim), horizontal pool with a stride-2 add, the
per-channel affine on the scalar engine (scale = weight/4 with the
pool normalization folded in), and a 1 MiB store issued from the sync
ring (same-window ~30 us/pass faster than gpsimd/SWDGE or scalar-ring
stores).

Measured by For_i delta-timing (kernel body repeated 257x in a
hardware loop, wall-time difference vs a single-pass NEFF; absolute
times drift ~15% between sessions on this shared device, so configs
were always compared within one process): this kernel 515-520 us/pass
vs 780-920 us for the previous strided-load kernel same-window
(~1.5x); best window observed 405 us/pass. The read stream runs at
~330 GB/s/core sustained with all 8 cores loaded (one core alone
reads at 420 GB/s, so this is the aggregate HBM ceiling), with the
33.6 MB/core of output writes largely overlapped.
"""

import numpy as np

import concourse.bacc as bacc
import concourse.bass as bass
import concourse.mybir as mybir
import concourse.tile as tile
from concourse.bass_utils import run_bass_kernel_spmd

N_CORES = 8
B, C, S = 16, 64, 512
B_LOC = B // N_CORES            # 2 batches per core
P = B_LOC * C                   # 128 partitions = one image per partition
IMG = S * S                     # 262144 input elems per image
OS = S // 2                     # 256
OUT_IMG = OS * OS               # 65536 output elems per image

ROWS = 16                       # input rows per iteration
CHUNK = ROWS * S                # 8192 elems per partition per load
N_ITERS = IMG // CHUNK          # 32
OUT_CHUNK = CHUNK // 4          # 2048 elems per partition per store

FP32 = mybir.dt.float32

_nc_cache = None


def _build(ibufs=4, vbufs=2, hbufs=2, obufs=2,
           load_engs=("sync", "scalar"), store_eng="sync", loop_n=0):
    # Bacc (not raw Bass): its finalize pass splits multi-sem waits into
    # event-semaphore instructions — TRN2 allows at most 1 wait per inst.
    nc = bacc.Bacc("TRN2", target_bir_lowering=False, debug=False,
                   num_devices=N_CORES)

    # x/out in iteration-major dense layouts (host transposes both ways)
    x = nc.declare_dram_parameter("x", [N_ITERS * P, CHUNK], FP32,
                                  isOutput=False)
    # affine[:, 0] = weight[c] / 4 (pool norm folded in), [:, 1] = bias[c]
    affine = nc.declare_dram_parameter("affine", [P, 2], FP32,
                                       isOutput=False)
    out = nc.declare_dram_parameter("out", [P, OUT_IMG], FP32,
                                    isOutput=True)

    engs = {"sync": nc.sync, "scalar": nc.scalar, "gpsimd": nc.gpsimd}
    ld = [engs[e] for e in load_engs]
    st = engs[store_eng]

    with tile.TileContext(nc) as tc:
        with tc.tile_pool(name="consts", bufs=1) as cpool, \
             tc.tile_pool(name="ld", bufs=ibufs) as ipool, \
             tc.tile_pool(name="vmid", bufs=vbufs) as vpool, \
             tc.tile_pool(name="hmid", bufs=hbufs) as hpool, \
             tc.tile_pool(name="st", bufs=obufs) as opool:

            cb = cpool.tile([P, 2], FP32)
            nc.sync.dma_start(out=cb[:], in_=affine[:, :])
            s_ap = cb[:, 0:1]
            b_ap = cb[:, 1:2]

            import contextlib
            loop_ctx = tc.For_i(0, loop_n, 1) if loop_n else \
                contextlib.nullcontext()
            with loop_ctx:
                for i in range(N_ITERS):
                    t = ipool.tile([P, CHUNK], FP32)
                    ld[i % len(ld)].dma_start(out=t[:],
                                              in_=x[i * P:(i + 1) * P, :])

                    # vertical pool: rows 2r and 2r+1 are adjacent spans
                    # in the free dim -> contiguous-stride add
                    tv = t[:].rearrange("p (r two w) -> p r two w",
                                        two=2, w=S)
                    v = vpool.tile([P, CHUNK // 2], FP32, name="v", tag="v")
                    vv = v[:].rearrange("p (r w) -> p r w", w=S)
                    nc.vector.tensor_add(vv, tv[:, :, 0, :], tv[:, :, 1, :])

                    # horizontal pool: adjacent column pairs, stride-2 ops
                    vh = v[:].rearrange("p (r j two) -> p r j two",
                                        two=2, j=OS)
                    h = hpool.tile([P, OUT_CHUNK], FP32, name="h", tag="h")
                    hh = h[:].rearrange("p (r j) -> p r j", j=OS)
                    nc.vector.tensor_add(hh, vh[:, :, :, 0], vh[:, :, :, 1])

                    # per-channel affine: y = Identity(h * (w[c]/4) + b[c])
                    y = opool.tile([P, OUT_CHUNK], FP32)
                    nc.scalar.activation(y[:], h[:],
                                         mybir.ActivationFunctionType.Identity,
                                         bias=b_ap, scale=s_ap)

                    st.dma_start(
                        out=out[:, i * OUT_CHUNK:(i + 1) * OUT_CHUNK],
                        in_=y[:])

    nc.finalize()
    return nc


def _get_nc():
    global _nc_cache
    if _nc_cache is None:
        _nc_cache = _build()
    return _nc_cache


def _make_in_maps(x, weight, bias):
    x = np.asarray(x, dtype=np.float32)
    weight = np.asarray(weight, dtype=np.float32).reshape(C)
    bias = np.asarray(bias, dtype=np.float32).reshape(C)
    affine = np.stack([np.tile(weight * 0.25, B_LOC),
                       np.tile(bias, B_LOC)], axis=1)
    affine = np.ascontiguousarray(affine, dtype=np.float32)  # [P, 2]
    in_maps = []
    for k in range(N_CORES):
        shard = x[k * B_LOC:(k + 1) * B_LOC].reshape(P, N_ITERS, CHUNK)
        shard = np.ascontiguousarray(shard.transpose(1, 0, 2)).reshape(
            N_ITERS * P, CHUNK)
        in_maps.append({"x": shard, "affine": affine})
    return in_maps


def run_sharded(x, weight, bias, trace=False, build_kw=None, **kw):
    """Run the SPMD kernel; returns (full_output, BassKernelResults)."""
    nc = _build(**build_kw) if build_kw else _get_nc()
    res = run_bass_kernel_spmd(nc, _make_in_maps(x, weight, bias),
                               core_ids=list(range(N_CORES)), trace=trace,
                               **kw)
    outs = [res.results[k]["out"].reshape(B_LOC, C, OS, OS)
            for k in range(N_CORES)]
    return np.concatenate(outs, axis=0), res


def kernel(x, weight, bias):
    out, _ = run_sharded(x, weight, bias, trace=False)
    return out
